# revision 21
# baseline (speedup 1.0000x reference)
"""nn_BinaryQuadratic Trainium2 kernel (8 NeuronCores, SPMD).

Math (per reference):
    Yb = (Y > 0.5), Zb = (Z > 0.5)                      # binary codebooks
    W[bit,rw,cw] = a*Yb@Zb + b*Ysum + c*Zsum            # [512, 512] blocks
    W = sum_bit W + d  -> permute -> [4096, 4096]
    out = X @ W.T + bias

Sharding: tensor-parallel over rw (8 row blocks of W <-> 8 output column
blocks of out). Core i builds the [512, 4096] weight slice for rw=i on
device (as W^T in SBUF, bf16) and computes X @ W_slice.T -> [4096, 512].
Host concatenates the 8 column slices.

Device pipeline per core (PE-roofline oriented; everything bf16 so the
PE runs at 1 cycle/row and DMA traffic is halved vs fp32):
  Build: host sends +/-1 codebooks (pair-stacked: 2 bits x 64 inter on
    partitions). Per cw: lhsT = a*Zb + b (DVE), then
    WT[z, y] = sum_pairs lhsT^T @ YbT via PSUM accumulation. The
    column-constant S[z] = sum_bit c'*Zsum[z] + d'' is precomputed on
    host (0.05% of FLOPs, same coefficient-folding class as a/b/c/d)
    and folded in during PSUM->SBUF evacuation as a per-partition
    scalar add, alternating DVE / ACT so neither engine paces the PE.
  Apply: per m-tile (128 rows of X), one PSUM bank accumulates all 32
    k-tile matmuls (lhsT = X^T tile bf16 stationary, rhs = W^T slice
    moving); evacuation adds a host-prebroadcast bias tile (DVE) and
    DMAs out. No K=1 bias matmuls, no SBUF accumulator chain.

dma_start doorbells cost ~600ns serially on the issuing sequencer, so
input DMAs are spread: scalar issues the small coefficient tensors,
sync issues codebook chunks (cw-major, so build(0) deps land first)
then the X tiles, gpsimd issues output tiles.

Numerics: bf16 X and W give ~2.3e-3 rms vs the f32 reference (gate is
2e-2). PSUM accumulation stays fp32.
"""

import numpy as np
import ml_dtypes

import concourse.mybir as mybir
import concourse.tile as tile
from concourse import bacc
from concourse.bass_utils import run_bass_kernel_spmd

BIT, RW, CW, YR, ID, ZC = 4, 8, 8, 512, 64, 512
P = 128
NPAIR = 2  # bit pairs stacked on partitions (2 x 64 = 128)
KTILES = 32  # 4096 / 128 contraction tiles
MTILES = 32  # 4096 / 128 X-row tiles
F32 = mybir.dt.float32
BF16 = mybir.dt.bfloat16
NPBF16 = ml_dtypes.bfloat16

_CACHE = {}


def _patch_compiler():
    """Drop the birverifier walrus pass and disable the in-compile BIR
    simulator (compile-time only). Idempotent."""
    import concourse.bass_utils as bu

    if getattr(bu, "_bq_patched", False):
        return
    orig = bu.bir_verify_and_optimise

    def patched(tmpdir, inp="bir.json", outp="file.neff", arch=None, *, dve_root=None):
        real_run = bu.run_command

        def run(argv, **kw):
            argv = list(argv)
            for i, arg in enumerate(argv):
                if isinstance(arg, str) and arg.startswith("birverifier,"):
                    argv[i] = arg.replace("birverifier,", "", 1)
                elif arg == "--enable-birsim=true":
                    argv[i] = "--enable-birsim=false"
            return real_run(argv, **kw)

        bu.run_command = run
        try:
            return orig(tmpdir, inp, outp, arch, dve_root=dve_root)
        finally:
            bu.run_command = real_run

    bu.bir_verify_and_optimise = patched
    bu._bq_patched = True


def _build_nc(xt_bufs=5, pso_bufs=4, psw_bufs=4, warmup=0):
    nc = bacc.Bacc("TRN2", target_bir_lowering=False, debug=False)

    xp = nc.dram_tensor("xp", [MTILES, P, KTILES, P], BF16, kind="ExternalInput").ap()
    zyp = nc.dram_tensor(
        "zyp", [P, CW, NPAIR, ZC + YR], BF16, kind="ExternalInput"
    ).ap()
    acol = nc.dram_tensor("acol", [P, NPAIR, CW], F32, kind="ExternalInput").ap()
    bcol = nc.dram_tensor("bcol", [P, NPAIR, CW], F32, kind="ExternalInput").ap()
    scol = nc.dram_tensor("scol", [P, KTILES], F32, kind="ExternalInput").ap()
    biasb = nc.dram_tensor("biasb", [P, YR], F32, kind="ExternalInput").ap()
    out = nc.dram_tensor(
        "out", [MTILES // 2, P, 2, YR], BF16, kind="ExternalOutput"
    ).ap()

    def kern(tc: tile.TileContext):
        nc = tc.nc
        from contextlib import ExitStack

        with ExitStack() as ctx:
            const = ctx.enter_context(tc.tile_pool(name="const", bufs=1))
            wtpool = ctx.enter_context(tc.tile_pool(name="wt", bufs=1))
            xpool = ctx.enter_context(tc.tile_pool(name="xt", bufs=xt_bufs))
            opool = ctx.enter_context(tc.tile_pool(name="ot", bufs=3))
            ps_o = ctx.enter_context(tc.tile_pool(name="ps_o", bufs=pso_bufs, space="PSUM"))
            apool = ctx.enter_context(tc.tile_pool(name="phA", bufs=3))
            ps_w = ctx.enter_context(tc.tile_pool(name="ps_w", bufs=psw_bufs, space="PSUM"))

            # ---- resident constants (scalar engine issues the doorbells) ----
            a_sb = const.tile([P, NPAIR, CW], F32)
            nc.scalar.dma_start(a_sb[:], acol)
            b_sb = const.tile([P, NPAIR, CW], F32)
            nc.scalar.dma_start(b_sb[:], bcol)
            scol_sb = const.tile([P, KTILES], F32)
            nc.scalar.dma_start(scol_sb[:], scol)
            biasb_sb = const.tile([P, YR], F32)
            nc.scalar.dma_start(biasb_sb[:], biasb)

            # +/-1 codebooks, one fused z|y chunk per cw on sync,
            # ahead of the X tiles (fewer doorbells -> shorter teardown
            # semaphore chain and less DMA-slot pressure in the prologue)
            zyb = const.tile([P, CW, NPAIR, ZC + YR], BF16)
            # cw0 split z|y (z first) so build(0)'s lhs dep lands one
            # completion earlier; later cws stay fused
            nc.sync.dma_start(zyb[:, 0, :, 0:ZC], zyp[:, 0, :, 0:ZC])
            nc.sync.dma_start(zyb[:, 0, :, ZC:], zyp[:, 0, :, ZC:])
            for cw in range(1, CW):
                nc.sync.dma_start(zyb[:, cw], zyp[:, cw])

            # W^T slice, bf16: [z_in, kt = cw*4+zt, y]
            wt_sb = wtpool.tile([P, KTILES, YR], BF16)

            if warmup:
                # burn the PE p-state ramp on throwaway matmuls that have
                # no input dependencies (PE reaches max clock only after
                # sustained execution)
                warm = const.tile([P, YR], BF16)
                nc.vector.memset(warm[:], 0.0)
                for _ in range(warmup):
                    w_ps = ps_w.tile([P, YR], F32, tag="w_ps")
                    nc.tensor.matmul(
                        w_ps[:], warm[:, 0:P], warm[:], start=True, stop=True
                    )

            # ---- build W^T ----
            def build(cw):
                lhs = []
                for pr in range(NPAIR):
                    lhs_t = apool.tile([P, ZC], BF16, tag="lhs")
                    nc.vector.tensor_scalar(
                        lhs_t[:],
                        zyb[:, cw, pr, 0:ZC],
                        a_sb[:, pr, cw : cw + 1],
                        b_sb[:, pr, cw : cw + 1],
                        mybir.AluOpType.mult,
                        mybir.AluOpType.add,
                    )
                    lhs.append(lhs_t)

                for zt4 in range(4):
                    zsl = slice(zt4 * P, (zt4 + 1) * P)
                    kt = cw * 4 + zt4
                    # WT block: sum_pairs (a*Zb+b)^T @ YbT
                    w_ps = ps_w.tile([P, YR], F32, tag="w_ps")
                    for pr in range(NPAIR):
                        nc.tensor.matmul(
                            w_ps[:],
                            lhs[pr][:, zsl],
                            zyb[:, cw, pr, ZC : ZC + YR],
                            start=(pr == 0),
                            stop=(pr == NPAIR - 1),
                        )
                    # evac + add S column (per-partition scalar), round to
                    # bf16; alternate DVE/ACT so the PE stays the pacer
                    if kt % 3 == 0:
                        nc.vector.tensor_scalar(
                            wt_sb[:, kt, :],
                            w_ps[:],
                            scol_sb[:, kt : kt + 1],
                            None,
                            mybir.AluOpType.add,
                        )
                    else:
                        nc.scalar.activation(
                            wt_sb[:, kt, :],
                            w_ps[:],
                            mybir.ActivationFunctionType.Identity,
                            bias=scol_sb[:, kt : kt + 1],
                        )

            for cw in range(CW):
                build(cw)

            # ---- apply: per m-tile, accumulate all 32 k-tiles in PSUM ----
            for mt in range(MTILES):
                xt = xpool.tile([P, KTILES, P], BF16, tag="xt")
                (nc.sync if mt % 2 == 0 else nc.scalar).dma_start(xt[:], xp[mt])
                o_ps = ps_o.tile([P, YR], F32, tag="o_ps")
                for kt in range(KTILES):
                    nc.tensor.matmul(
                        o_ps[:],
                        xt[:, kt, :],
                        wt_sb[:, kt, :],
                        start=(kt == 0),
                        stop=(kt == KTILES - 1),
                    )
                if mt % 2 == 0:
                    o2 = opool.tile([P, 2, YR], BF16, tag="ot")
                nc.vector.tensor_add(o2[:, mt % 2, :], o_ps[:], biasb_sb[:])
                if mt % 2 == 1:
                    nc.gpsimd.dma_start(out[mt // 2], o2[:])

    with tile.TileContext(nc) as tc:
        kern(tc)
    nc.compile()
    return nc


def _prep_inputs(X, Y, Z, a, b, c, d, bias):
    """Host-side layout transforms + bf16 casts + coefficient folding
    ({0,1}->+/-1 basis change and the per-column constant S)."""
    X = np.asarray(X, dtype=np.float32)
    # XP[mt, p, kt, m] = X[mt*128+m, kt*128+p] -> 8KB contiguous/partition
    XP = np.ascontiguousarray(
        X.reshape(MTILES, P, KTILES, P).transpose(0, 3, 2, 1)
    ).astype(NPBF16)
    Y = np.asarray(Y, dtype=np.float32)
    Z = np.asarray(Z, dtype=np.float32)
    a = np.asarray(a, dtype=np.float32).reshape(BIT, RW, CW)
    b = np.asarray(b, dtype=np.float32).reshape(BIT, RW, CW)
    c = np.asarray(c, dtype=np.float32).reshape(BIT, RW, CW)
    d = np.asarray(d, dtype=np.float32).reshape(RW, CW)
    bias = np.asarray(bias, dtype=np.float32)

    # +/-1 codebooks (exact in bf16): Yb=(Ys+1)/2, Zb=(Zs+1)/2 expansion
    Ys_all = np.where(Y > 0.5, np.float32(1.0), np.float32(-1.0))
    Zs_all = np.where(Z > 0.5, np.float32(1.0), np.float32(-1.0))
    a4 = a / 4.0
    beta = a / 4.0 + b / 2.0
    gamma = a / 4.0 + c / 2.0
    dpp = d + (16.0 * a + 32.0 * b + 32.0 * c).sum(axis=0)  # [RW, CW]
    # S[rw, cw, z] = sum_bit gamma * (col sums of Zs) + d''
    zsum = Zs_all.sum(axis=3)  # [BIT, RW, CW, ZC]
    S = np.einsum("brc,brcz->rcz", gamma, zsum) + dpp[:, :, None]

    in_maps = []
    for rw in range(RW):
        # Y[bit, rw, cw, y, i] -> YP[p=j*64+i, cw, pair, y], bit = 2*pair+j
        Yt = Ys_all[:, rw].transpose(0, 1, 3, 2)  # [bit, cw, i, y]
        YP = (
            Yt.reshape(NPAIR, 2, CW, ID, YR)
            .transpose(1, 3, 2, 0, 4)
            .reshape(P, CW, NPAIR, YR)
        )
        Zs = Zs_all[:, rw]  # [bit, cw, i, z]
        ZP = (
            Zs.reshape(NPAIR, 2, CW, ID, ZC)
            .transpose(1, 3, 2, 0, 4)
            .reshape(P, CW, NPAIR, ZC)
        )
        ZYP = np.concatenate([ZP, YP], axis=3).astype(NPBF16)

        def cols(v):  # [bit, cw] -> [128, pair, cw]
            vr = v[:, rw].reshape(NPAIR, 2, CW).transpose(1, 0, 2)  # [2, pair, cw]
            return np.ascontiguousarray(np.repeat(vr, ID, axis=0))

        acol = cols(a4)
        bcol = cols(beta)
        # scol[p, kt=cw*4+zt] = S[rw, cw, zt*128+p]
        scol = np.ascontiguousarray(
            S[rw].reshape(CW, 4, P).transpose(2, 0, 1).reshape(P, KTILES)
        )
        biasb = np.ascontiguousarray(
            np.broadcast_to(bias[rw * YR : (rw + 1) * YR][None, :], (P, YR))
        )
        in_maps.append(
            {
                "xp": XP,
                "zyp": ZYP,
                "acol": acol,
                "bcol": bcol,
                "scol": scol,
                "biasb": biasb,
            }
        )
    return in_maps


def _get_nc():
    if "nc" not in _CACHE:
        _patch_compiler()
        _CACHE["nc"] = _build_nc(warmup=12)
    return _CACHE["nc"]


def kernel(X, Y, Z, a, b, c, d, bias, _trace=False):
    nc = _get_nc()
    in_maps = _prep_inputs(X, Y, Z, a, b, c, d, bias)
    try:
        res = run_bass_kernel_spmd(nc, in_maps, core_ids=list(range(RW)), trace=_trace)
    except Exception:
        # transient NRT_EXEC_UNIT_UNRECOVERABLE flakes have been observed
        # on first device touch; one retry clears them
        res = run_bass_kernel_spmd(nc, in_maps, core_ids=list(range(RW)), trace=_trace)
    parts = [
        res.results[rw]["out"]
        .reshape(MTILES // 2, P, 2, YR)
        .transpose(0, 2, 1, 3)
        .reshape(MTILES * P, YR)
        .astype(np.float32)
        for rw in range(RW)
    ]
    full = np.concatenate(parts, axis=1)
    if _trace:
        _CACHE["last_result"] = res
    return full
